# revision 23
# baseline (speedup 1.0000x reference)
"""BatchedGCN Trainium2 kernel (v4: symmetric gram + dual-engine A eviction).

Per graph (batch element):
  norms_i = ||X_i||;  A = (X@X.T > 0.3*n_i*n_j) + I ; deg = rowsum(A); d = deg^-1/2
  H1 = relu(diag(d) A diag(d) (X @ W1.T) + b1)
  H2 = diag(d) A diag(d) (H1 @ W2.T) + b2
  out = H2 / max(||H2_row||, 1e-12)

Key implementation choices:
- Threshold in un-normalized form: Xn_i . Xn_j > t  <=>
  (X_i . X_j) * (1/max(n_i,eps)) / t > n_j; the diag(norm) factor cancels
  against the un-normalized X in the first linear layer.
- Gram runs in fp8 DoubleRow (2x K per pass); thresholding margin >> fp8
  error so A is exact.
- SYMMETRY: only the upper triangle of G is computed (diag prepass for
  norms + row-major upper chunks).  The lower triangle of the thresholded
  A is mirrored via PE transposes (bf16 psum) and evicted alternately on
  the ACT and DVE engines (with deg accumulation), in parallel with the
  DVE thresholds of the upper chunks.  This cuts gram matmul time ~45%
  and splits the PSUM-eviction cost across two engines.
- deg accumulates into a 4-slot [128, 4*NT] layout (2 upper-chunk slots +
  2 mirror slots per row tile), summed once per graph.
- Queue discipline: input loads are per-tile on the sync queue (spreads
  DMA rings), fp8 for all graphs first.  All graphs' norm bounces issue
  before any graph's deg bounce (phase b1/b2 split) so a graph's bounce
  never queues behind the previous graph's full threshold pipeline.
- Batched phase-e evictions + output normalization with free-dim-
  broadcast APs; per-graph staged output with a single store DMA.
- All graphs on a core are emitted phase-major so each graph's latency
  chains hide behind other graphs' dense matmul phases.
"""

from contextlib import ExitStack

import ml_dtypes
import numpy as np

import concourse.bass as bass
import concourse.mybir as mybir
import concourse.tile as tile
from concourse import bacc
from concourse.bass_utils import run_bass_kernel_spmd
from concourse.masks import make_identity

B, N, D_IN, D_H, D_OUT = 32, 1024, 768, 256, 128
N_CORES = 8
BPC = B // N_CORES          # graphs per core
NT = N // 128               # 8 row tiles
DTI = D_IN // 128           # 6 input-dim tiles
HC = D_H // 128             # 2 hidden chunks
KDR = D_IN // 256           # 3 DoubleRow K-chunks
F32 = mybir.dt.float32
BF16 = mybir.dt.bfloat16
FP8 = mybir.dt.float8e4

KNN_THRESHOLD = 0.3
COS_EPS = 1e-8
NORM_EPS = 1e-12
ALU = mybir.AluOpType
AF = mybir.ActivationFunctionType
DR = mybir.MatmulPerfMode.DoubleRow


def build(n_batches: int = BPC):
    nc = bacc.Bacc("TRN2", debug=False, num_devices=N_CORES)
    XT = nc.dram_tensor("XT", [n_batches, D_IN, N], BF16, kind="ExternalInput")
    # X^T in fp8, pair-interleaved for DoubleRow: [b, k, p, i, n] with
    # d = k*256 + i*128 + p
    XT8 = nc.dram_tensor("XT8", [n_batches, KDR, 128, 2, N], FP8,
                         kind="ExternalInput")
    W1T = nc.dram_tensor("W1T", [D_IN, D_H], BF16, kind="ExternalInput")
    b1 = nc.dram_tensor("b1", [D_H], F32, kind="ExternalInput")
    W2T = nc.dram_tensor("W2T", [D_H, D_OUT], BF16, kind="ExternalInput")
    b2 = nc.dram_tensor("b2", [D_OUT], F32, kind="ExternalInput")
    Y = nc.dram_tensor("Y", [n_batches, N, D_OUT], F32, kind="ExternalOutput")
    with tile.TileContext(nc) as tc, ExitStack() as ctx:
        _body(ctx, tc, XT.ap(), XT8.ap(), W1T.ap(), b1.ap(), W2T.ap(), b2.ap(),
              Y.ap(), n_batches)
    nc.compile()
    return nc


def _bcast_p(ap: bass.AP, parts: int = 128) -> bass.AP:
    """Broadcast a DRAM AP across `parts` partitions (partition-stride 0)."""
    return bass.AP(tensor=ap.tensor, offset=ap.offset, ap=[[0, parts]] + list(ap.ap))


class _GraphState:
    """Per-graph SBUF tiles threaded between pipeline phases."""
    __slots__ = ("bi", "xt8g", "xtg", "at", "ys1", "ys2g", "h1t", "ncol",
                 "ssqv", "rc03", "nrep", "degv", "dv", "drep", "og", "s2col")


def _body(ctx, tc, XT, XT8, W1T, b1, W2T, b2, Y, n_batches):
    nc = tc.nc

    nb = n_batches
    singles = ctx.enter_context(tc.tile_pool(name="singles", bufs=1))
    sqj = ctx.enter_context(tc.tile_pool(name="sqj", bufs=2))
    xtpool = ctx.enter_context(tc.tile_pool(name="xtpool", bufs=1))
    apool = ctx.enter_context(tc.tile_pool(name="apool", bufs=nb * NT))
    bvec = ctx.enter_context(tc.tile_pool(name="bvec", bufs=nb))
    y1pool = ctx.enter_context(tc.tile_pool(name="y1pool", bufs=nb * NT))
    h1pool = ctx.enter_context(tc.tile_pool(name="h1pool", bufs=3 * HC))
    y2pool = ctx.enter_context(tc.tile_pool(name="y2pool", bufs=4))
    rppool = ctx.enter_context(tc.tile_pool(name="rppool", bufs=nb))
    tmppool = ctx.enter_context(tc.tile_pool(name="tmppool", bufs=4))
    opool = ctx.enter_context(tc.tile_pool(name="opool", bufs=2))
    psA = ctx.enter_context(tc.tile_pool(name="psA", bufs=4, space="PSUM"))
    psB = ctx.enter_context(tc.tile_pool(name="psB", bufs=3, space="PSUM"))
    psT = ctx.enter_context(tc.tile_pool(name="psT", bufs=1, space="PSUM"))
    dramp = ctx.enter_context(tc.tile_pool(name="dramp", bufs=nb, space="DRAM"))

    # ---- one-time constants (plain loads, no prep chains) -------------------
    ident = singles.tile([128, 128], BF16)
    make_identity(nc, ident)
    identf = singles.tile([128, 128], F32)
    make_identity(nc, identf)

    b1col = singles.tile([128, HC], F32)
    nc.sync.dma_start(out=b1col, in_=bass.AP(tensor=b1.tensor, offset=b1.offset,
                                             ap=[[1, 128], [128, HC]]))
    b2rep = singles.tile([128, D_OUT], F32)
    nc.gpsimd.dma_start(out=b2rep, in_=_bcast_p(b2))

    w1t = []
    for dt in range(DTI):
        t = singles.tile([128, D_H], BF16, tag=f"w1t{dt}")
        nc.sync.dma_start(out=t, in_=W1T[dt * 128:(dt + 1) * 128, :])
        w1t.append(t)
    w2t = []
    for k in range(HC):
        t = singles.tile([128, D_OUT], BF16, tag=f"w2t{k}")
        nc.sync.dma_start(out=t, in_=W2T[k * 128:(k + 1) * 128, :])
        w2t.append(t)

    inv_t = 1.0 / KNN_THRESHOLD

    # ---- per-phase emitters -------------------------------------------------
    def phase_a(g: _GraphState):
        # merged fp8 input load: one DMA per graph (descriptors spread over
        # all rings; single cheap issuance on the sync sequencer), all
        # graphs before any bf16 load
        g.xt8g = xtpool.tile([128, KDR, 2, N], FP8, tag="xt8g", bufs=nb)
        src8 = bass.AP(
            tensor=XT8.tensor, offset=XT8.offset + g.bi * (KDR * 128 * 2 * N),
            ap=[[2 * N, 128], [128 * 2 * N, KDR], [N, 2], [1, N]])
        nc.sync.dma_start(out=g.xt8g, in_=src8)
        g.at = []
        for it in range(NT):
            g.at.append(apool.tile([128, N], BF16, tag="a_t", name="a_t"))
        g.ys1 = []

    def phase_a2(g: _GraphState):
        # merged bf16 input load (only consumed by phase_c)
        g.xtg = xtpool.tile([128, DTI, N], BF16, tag="xtg", bufs=2)
        srcb = bass.AP(
            tensor=XT.tensor, offset=XT.offset + g.bi * (D_IN * N),
            ap=[[N, 128], [128 * N, DTI], [1, N]])
        nc.sync.dma_start(out=g.xtg, in_=srcb)

    def phase_b1(g: _GraphState):
        # diag prepass: row norms from the gram diagonal blocks; issue the
        # norm bounce for every graph before any graph's deg bounce
        g.ssqv = bvec.tile([128, NT], F32, tag="ssqv")
        for h in range(2):
            psd = psB.tile([128, 512], F32, tag="psB", name="psd")
            for q in range(4):
                it = 4 * h + q
                blk = slice(it * 128, (it + 1) * 128)
                for k in range(KDR):
                    nc.tensor.matmul(psd[:, q * 128:(q + 1) * 128],
                                     lhsT=g.xt8g[:, k, :, blk],
                                     rhs=g.xt8g[:, k, :, blk],
                                     start=(k == 0), stop=(k == KDR - 1),
                                     perf_mode=DR)
            for q in range(4):
                it = 4 * h + q
                dj = sqj.tile([128, 128], BF16, tag="dj")
                nc.vector.scalar_tensor_tensor(
                    out=dj, in0=psd[:, q * 128:(q + 1) * 128], scalar=1.0,
                    in1=identf, op0=ALU.bypass, op1=ALU.mult,
                    accum_out=g.ssqv[:, it:it + 1])

        g.ncol = bvec.tile([128, NT], F32, tag="ncol")
        nc.scalar.sqrt(out=g.ncol, in_=g.ssqv)

        # bounce ncol -> DRAM -> Nrep (n_j replicated over partitions, bf16)
        nscr = dramp.tile([1, N], F32, tag="nscr")
        nflat = nscr[0]
        nc.gpsimd.dma_start(
            out=bass.AP(tensor=nflat.tensor, offset=nflat.offset,
                        ap=[[1, 128], [128, NT]]),
            in_=g.ncol)
        g.nrep = rppool.tile([128, N], BF16, tag="nrep")
        nc.gpsimd.dma_start(out=g.nrep, in_=_bcast_p(nflat))

    def phase_b2(g: _GraphState):
        # per-graph scalar chain emitted here (not in b1) so the DVE queue
        # never stalls on a later graph's sqrt before this graph's
        # thresholds can run
        nclamp = bvec.tile([128, NT], F32, tag="nclamp")
        nc.vector.tensor_scalar_max(nclamp, g.ncol, COS_EPS)
        rcol = bvec.tile([128, NT], F32, tag="rcol")
        nc.vector.reciprocal(out=rcol, in_=nclamp)
        g.rc03 = bvec.tile([128, NT], F32, tag="rc03")
        nc.vector.tensor_scalar_mul(g.rc03, rcol, inv_t)

        # upper-triangle row chunks: G -> threshold -> A, deg fused
        g.degv = bvec.tile([128, 4 * NT], F32, tag="degv")
        nc.vector.memset(g.degv, 0.0)
        for it in range(NT):
            ci = 0
            for c0 in range(it * 128, N, 512):
                w = min(512, N - c0)
                ps = psA.tile([128, 512], F32, tag="psA")
                for k in range(KDR):
                    nc.tensor.matmul(ps[:, :w],
                                     lhsT=g.xt8g[:, k, :,
                                                 it * 128:(it + 1) * 128],
                                     rhs=g.xt8g[:, k, :, c0:c0 + w],
                                     start=(k == 0), stop=(k == KDR - 1),
                                     perf_mode=DR)
                nc.vector.scalar_tensor_tensor(
                    out=g.at[it][:, c0:c0 + w], in0=ps[:, :w],
                    scalar=g.rc03[:, it:it + 1],
                    in1=g.nrep[:, c0:c0 + w],
                    op0=ALU.mult, op1=ALU.is_gt,
                    accum_out=g.degv[:, ci * NT + it:ci * NT + it + 1])
                ci += 1
            # self loop (diag becomes 2); deg handled via sqrt bias
            nc.vector.tensor_tensor(
                out=g.at[it][:, it * 128:(it + 1) * 128],
                in0=g.at[it][:, it * 128:(it + 1) * 128],
                in1=ident, op=ALU.add)

        # mirror the lower triangle: PE transpose, ACT/DVE eviction
        mg = 0
        for jt in range(1, NT):
            ci = 2
            for c0 in range(0, jt * 128, 512):
                w = min(512, jt * 128 - c0)
                pst = psT.tile([128, 512], BF16, tag="psT")
                for bq in range(w // 128):
                    it = (c0 + bq * 128) // 128
                    nc.tensor.transpose(
                        pst[:, bq * 128:(bq + 1) * 128],
                        g.at[it][:, jt * 128:(jt + 1) * 128], ident)
                acc = g.degv[:, ci * NT + jt:ci * NT + jt + 1]
                if mg % 3 != 2:
                    nc.scalar.activation(
                        out=g.at[jt][:, c0:c0 + w], in_=pst[:, :w],
                        func=AF.Copy, accum_out=acc)
                else:
                    nc.vector.tensor_scalar(
                        out=g.at[jt][:, c0:c0 + w], in0=pst[:, :w],
                        scalar1=0.0, scalar2=1.0, op0=ALU.add, op1=ALU.mult,
                        accum_out=acc)
                mg += 1
                ci += 1

        # deg -> d = (deg+1)^-1/2 -> Drep bounce
        dsum = bvec.tile([128, NT], F32, tag="dsum")
        nc.vector.tensor_tensor(out=dsum, in0=g.degv[:, 0:NT],
                                in1=g.degv[:, NT:2 * NT], op=ALU.add)
        dsum2 = bvec.tile([128, NT], F32, tag="dsum2")
        nc.vector.tensor_tensor(out=dsum2, in0=g.degv[:, 2 * NT:3 * NT],
                                in1=g.degv[:, 3 * NT:4 * NT], op=ALU.add)
        dsum3 = bvec.tile([128, NT], F32, tag="dsum3")
        nc.vector.tensor_tensor(out=dsum3, in0=dsum, in1=dsum2, op=ALU.add)
        sqd = bvec.tile([128, NT], F32, tag="sqd")
        nc.scalar.activation(out=sqd, in_=dsum3, func=AF.Sqrt, bias=1.0)
        g.dv = bvec.tile([128, NT], F32, tag="dv")
        nc.vector.reciprocal(out=g.dv, in_=sqd)

        dscr = dramp.tile([1, N], F32, tag="dscr")
        dflat = dscr[0]
        nc.gpsimd.dma_start(
            out=bass.AP(tensor=dflat.tensor, offset=dflat.offset,
                        ap=[[1, 128], [128, NT]]),
            in_=g.dv)
        g.drep = rppool.tile([128, N], BF16, tag="drep")
        nc.gpsimd.dma_start(out=g.drep, in_=_bcast_p(dflat))

    def phase_c(g: _GraphState):
        # G1 = X @ W1.T [n, h]; evict scaled by d -> Ys1 bf16 (ACT engine)
        for it in range(NT):
            ps = psB.tile([128, 512], F32, tag="psB", name="psc")
            for dt in range(DTI):
                nc.tensor.matmul(ps[:, :D_H],
                                 lhsT=g.xtg[:, dt, it * 128:(it + 1) * 128],
                                 rhs=w1t[dt], start=(dt == 0),
                                 stop=(dt == DTI - 1))
            y1 = y1pool.tile([128, D_H], BF16, tag="y1")
            nc.scalar.activation(out=y1, in_=ps[:, :D_H], func=AF.Copy,
                                 scale=g.dv[:, it:it + 1])
            g.ys1.append(y1)

    def phase_d(g: _GraphState):
        # M1^T = (A diag(d) G1)^T over 4 concurrent PSUM groups (hc x ih),
        # K-contiguous in jt; H1^T = relu(d_i * M1^T + b1)
        g.h1t = []
        pss = {}
        for hc in range(HC):
            g.h1t.append(h1pool.tile([128, N], BF16, tag="h1", name="h1"))
            for ih in range(2):
                pss[hc, ih] = psA.tile([128, 512], F32, tag="psA", name="psd2")
        for jt in range(NT):
            st = jt == 0
            sp = jt == NT - 1
            for hc in range(HC):
                lhsT = g.ys1[jt][:, hc * 128:(hc + 1) * 128]
                for ih in range(2):
                    nc.tensor.matmul(pss[hc, ih], lhsT=lhsT,
                                     rhs=g.at[jt][:, ih * 512:(ih + 1) * 512],
                                     start=st, stop=sp)
        for hc in range(HC):
            for ih in range(2):
                tmp = tmppool.tile([128, 512], F32, tag="tmp")
                nc.vector.tensor_tensor(out=tmp, in0=pss[hc, ih],
                                        in1=g.drep[:, ih * 512:(ih + 1) * 512],
                                        op=ALU.mult)
                nc.scalar.activation(out=g.h1t[hc][:, ih * 512:(ih + 1) * 512],
                                     in_=tmp, func=AF.Relu,
                                     bias=b1col[:, hc:hc + 1])

    def phase_e(g: _GraphState):
        # Y2 = d * (H1 @ W2.T): batched over 4 row tiles per psum bank,
        # evicted with a free-dim-broadcast multiply by d
        g.ys2g = []
        for half in range(2):
            ps = psB.tile([128, 512], F32, tag="psB", name="pse")
            for q in range(4):
                it = 4 * half + q
                for hc in range(HC):
                    nc.tensor.matmul(ps[:, q * 128:(q + 1) * 128],
                                     lhsT=g.h1t[hc][:, it * 128:(it + 1) * 128],
                                     rhs=w2t[hc], start=(hc == 0),
                                     stop=(hc == HC - 1))
            y2 = y2pool.tile([128, 512], BF16, tag="y2")
            ps3 = bass.AP(tensor=ps.tensor, offset=ps.offset,
                          ap=[[512, 128], [128, 4], [1, 128]])
            y23 = bass.AP(tensor=y2.tensor, offset=y2.offset,
                          ap=[[512, 128], [128, 4], [1, 128]])
            dv3 = bass.AP(tensor=g.dv.tensor, offset=g.dv.offset + 4 * half,
                          ap=[[NT, 128], [1, 4], [0, 128]])
            nc.vector.tensor_tensor(out=y23, in0=ps3, in1=dv3, op=ALU.mult)
            g.ys2g.append(y2)

    def phase_f(g: _GraphState):
        # H2 = d * (A @ Y2) + b2, row-normalized, staged then stored once
        g.og = opool.tile([128, N], F32, tag="og")
        g.s2col = bvec.tile([128, NT], F32, tag="s2col")
        for it in range(NT):
            ps = psB.tile([128, 512], F32, tag="psB", name="psf")
            for jt in range(NT):
                nc.tensor.matmul(ps[:, :D_OUT],
                                 lhsT=g.at[jt][:, it * 128:(it + 1) * 128],
                                 rhs=g.ys2g[jt // 4][:, (jt % 4) * 128:
                                                    (jt % 4 + 1) * 128],
                                 start=(jt == 0), stop=(jt == NT - 1))
            nc.vector.scalar_tensor_tensor(
                out=g.og[:, it * 128:(it + 1) * 128], in0=ps[:, :D_OUT],
                scalar=g.dv[:, it:it + 1], in1=b2rep,
                op0=ALU.mult, op1=ALU.add)
            sj = sqj.tile([128, D_OUT], F32, tag="sqj")
            nc.scalar.activation(out=sj, in_=g.og[:, it * 128:(it + 1) * 128],
                                 func=AF.Square,
                                 accum_out=g.s2col[:, it:it + 1])
        nrm = bvec.tile([128, NT], F32, tag="nrm")
        nc.scalar.sqrt(out=nrm, in_=g.s2col)
        cl = bvec.tile([128, NT], F32, tag="cl")
        nc.vector.tensor_scalar_max(cl, nrm, NORM_EPS)
        inv2 = bvec.tile([128, NT], F32, tag="inv2")
        nc.vector.reciprocal(out=inv2, in_=cl)
        og3 = bass.AP(tensor=g.og.tensor, offset=g.og.offset,
                      ap=[[N, 128], [128, NT], [1, 128]])
        iv3 = bass.AP(tensor=inv2.tensor, offset=inv2.offset,
                      ap=[[NT, 128], [1, NT], [0, 128]])
        nc.vector.tensor_tensor(out=og3, in0=og3, in1=iv3, op=ALU.mult)
        dst = bass.AP(tensor=Y.tensor, offset=Y.offset + g.bi * (N * D_OUT),
                      ap=[[D_OUT, 128], [128 * D_OUT, NT], [1, D_OUT]])
        nc.gpsimd.dma_start(out=dst, in_=g.og)

    # ---- wave-pipelined driver: all graphs in flight, phase-major -----------
    gs = []
    for bi in range(n_batches):
        g = _GraphState()
        g.bi = bi
        gs.append(g)

    for g in gs:
        phase_a(g)
    for g in gs:
        phase_b1(g)
    # bf16 loads issue after all norm bounces: their ring descriptors would
    # otherwise delay the latency-critical nrep broadcasts by ~15us
    for g in gs:
        phase_a2(g)
    for g in gs:
        phase_b2(g)
    for g in gs:
        phase_c(g)
    for g in gs:
        phase_d(g)
        phase_e(g)
        phase_f(g)


_NC_CACHE = {}


def _get_nc(n_batches: int = BPC):
    if n_batches not in _NC_CACHE:
        _NC_CACHE[n_batches] = build(n_batches)
    return _NC_CACHE[n_batches]


def make_in_maps(X, W1, b1, W2, b2, bpc: int = BPC):
    X = np.asarray(X, dtype=np.float32)
    nb = len(X)
    Xt = X.astype(ml_dtypes.bfloat16).transpose(0, 2, 1)   # [B, D, N] bf16
    XTb16 = np.ascontiguousarray(Xt)
    # DoubleRow pair-interleaved fp8: [b, k, p, i, n], d = k*256 + i*128 + p
    XT8 = np.ascontiguousarray(
        Xt.reshape(nb, KDR, 2, 128, N).transpose(0, 1, 3, 2, 4)
        .astype(ml_dtypes.float8_e4m3))
    W1T = np.ascontiguousarray(
        np.asarray(W1, dtype=np.float32).T.astype(ml_dtypes.bfloat16))
    W2T = np.ascontiguousarray(
        np.asarray(W2, dtype=np.float32).T.astype(ml_dtypes.bfloat16))
    b1 = np.ascontiguousarray(np.asarray(b1, dtype=np.float32))
    b2 = np.ascontiguousarray(np.asarray(b2, dtype=np.float32))
    return [
        {"XT": XTb16[c * bpc:(c + 1) * bpc], "XT8": XT8[c * bpc:(c + 1) * bpc],
         "W1T": W1T, "b1": b1, "W2T": W2T, "b2": b2}
        for c in range(nb // bpc)
    ]


def kernel(X, W1, b1, W2, b2):
    nc = _get_nc()
    in_maps = make_in_maps(X, W1, b1, W2, b2)
    res = run_bass_kernel_spmd(nc, in_maps, core_ids=list(range(N_CORES)))
    return np.concatenate([r["Y"] for r in res.results], axis=0)
